# revision 41
# baseline (speedup 1.0000x reference)
"""TRN2 Bass/Tile kernel for the cosine-similarity attention block.

Reference math (fp32, single device):
    K = X @ Wk.T + Wk0 ; Q = X @ Wq.T + Wq0          # [N, E]
    Y = (Q @ K.T) / sqrt(max(|Q_m|^2 * |K_n|^2, eps)) # [N, N] cosine sims
    SM = softmax(Y, axis=0)                           # column softmax
    Z = SM @ X                                        # [N, E]

Distribution (8 cores, row-sharded): each core owns M = N/8 rows of Q /
output rows of Z. Everything heavy runs as fp8-e4m3 DoubleRow matmuls
(2 k-tiles per pass at 0.5 cyc/row; fp32 PSUM accumulation):

  Phase 0: K/Q projections from fp8 X^T/W^T; row norms via squares +
    ones-matmul reduction; x4 fp8 scale folded into the 1/norm
    broadcast. K's norm chain is emitted between Q matmul blocks so the
    AllGather launches early while the PE stays busy.
  Phase 1: AllGather of normalized K^T in fp8 (1 MB/rank).
  Phase 2: Yt slice via DoubleRow (psy holds 16*Y), exp on ScalarE with
    scale=1/16 and accum_out producing per-column partial sums for free
    (fp16 ring buffer). Column sums AllReduce in 4 chunks of 16 n-tiles
    so early chunks overlap the Y phase. Two phase-4 psz chains are
    interleaved into this ScalarE-bound window, lagging the AR by one
    chunk so the in-order PE queue never waits on a collective.
  Phase 3 (per AR chunk): v = (exp(y) - cs/8192) * (2^17/cs), i.e.
    2^17*(SM - 1/8192), folded on DVE (+ScalarE for the last chunk)
    into fp8 pair tiles. Centering before the fp8 quantization is what
    makes fp8 viable: softmax weights concentrate at 1/8192*(1 +- 0.12),
    so raw e4m3 (3 mantissa bits) would destroy the signal, while the
    centered residuals span binades. (GPSIMD is software-emulated on
    TRN2 -- never put bulk elementwise work there.)
  Phase 4: Zt = X^T-panels(fp8) @ V(fp8) via DoubleRow, then
    Zt = psz/2^17 + Sx/8192 with Sx = colsum of X (host-exact),
    restoring the centering: SM@X = (SM-c)@X + c*(1^T X), c = 1/8192.

The repeat loop covers all phases (per-rep AG/AR buffers) so a
repeat-NEFF slope measures honest steady-state per-exec device time.
Measured end-to-end error ~7.4e-3 scale-relative absmax (gate: 2e-2).
"""

import os
from contextlib import ExitStack

import numpy as np

N, E, C = 8192, 1024, 8

_CACHE = {}


def _build_program(n=N, e=E, c=C, solo=False, repeat=1, upto=4):
    """Emit + compile the SPMD Bass program (one NEFF, all cores)."""
    import concourse.bacc as bacc
    import concourse.mybir as mybir
    import concourse.tile as tile

    F32 = mybir.dt.float32
    F16 = mybir.dt.float16
    F8 = mybir.dt.float8e4
    AF = mybir.ActivationFunctionType
    PM = mybir.MatmulPerfMode.DoubleRow
    ALU = mybir.AluOpType

    m = n // c          # rows per core
    et = e // 128       # e-tiles
    nt = n // 128       # n-tiles
    jt = m // 128       # n-tiles per core block
    mch = [(i, min(512, m - i)) for i in range(0, m, 512)]  # m chunks (<=512)
    CH = 16             # AR chunk size in n-tiles
    nch = nt // CH
    EB = 32             # fp16 exp ring size (tiles)
    S_V = 131072.0      # 2^17: fp8 scale for centered softmax weights
    rg = [list(range(c))]

    nc = bacc.Bacc("TRN2", target_bir_lowering=False, debug=False, num_devices=c)

    xt = nc.dram_tensor("xt", [e, m], F8, kind="ExternalInput")
    wqt = nc.dram_tensor("wqt", [e, e], F8, kind="ExternalInput")
    wkt = nc.dram_tensor("wkt", [e, e], F8, kind="ExternalInput")
    bq = nc.dram_tensor("bq", [et, 128], F32, kind="ExternalInput")
    bk = nc.dram_tensor("bk", [et, 128], F32, kind="ExternalInput")
    xp = nc.dram_tensor("xp", [et, 128, nt, 128], F8, kind="ExternalInput")
    sx = nc.dram_tensor("sx", [et, 128], F32, kind="ExternalInput")  # X colsum/8192
    zt = nc.dram_tensor("zt", [e, m], F16, kind="ExternalOutput")

    with ExitStack() as ctx:
        tc = ctx.enter_context(tile.TileContext(nc))

        dram = ctx.enter_context(tc.tile_pool(name="dram", bufs=1, space="DRAM"))

        consts = ctx.enter_context(tc.tile_pool(name="consts", bufs=1))
        ones_k = consts.tile([128, 1], F8)
        ones4_m = consts.tile([1, 128], F16)
        nc.vector.memset(ones_k, 1.0)
        nc.vector.memset(ones4_m, 4.0)  # folds the x4 fp8 scale into Kn/Qn
        bias_q = consts.tile([128, et], F32)
        bias_k = consts.tile([128, et], F32)
        nc.sync.dma_start(bias_q, bq.ap().rearrange("t p -> p t"))
        nc.sync.dma_start(bias_k, bk.ap().rearrange("t p -> p t"))
        sxb = consts.tile([128, et], F32)
        nc.sync.dma_start(sxb, sx.ap().rearrange("t p -> p t"))
        eps1 = consts.tile([1, 1], F32)
        nc.vector.memset(eps1, 1e-6)
        colsum = consts.tile([128, nt], F32)
        cs_full = consts.tile([128, nt], F32)
        rec_cs = consts.tile([128, nt], F32)
        sfac = consts.tile([128, nt], F32)   # 2^17 / cs
        tfac = consts.tile([128, nt], F32)   # cs / 8192

        # persistent across reps: fp8 Qn^T [128, et, m], fp8 centered-SM
        # tiles in DoubleRow pair layout [128, 2, m], and the phase-0
        # working set (persistent tags let the next rep's input DMAs start
        # as soon as the previous rep's last reader is done, instead of
        # waiting for a whole pool region to free).
        qn_pool = ctx.enter_context(tc.tile_pool(name="qn", bufs=1))
        qn8 = [qn_pool.tile([128, 2, m], F8, tag=f"qn8_{p}", name=f"qn8_{p}")
               for p in range(et // 2)]
        et_pool = ctx.enter_context(tc.tile_pool(name="etp", bufs=1))
        ets8 = [et_pool.tile([128, 2, m], F8, tag=f"e8_{i}", name=f"e8_{i}")
                for i in range(nt // 2)]
        p0 = ctx.enter_context(tc.tile_pool(name="p0", bufs=1))
        x8 = [p0.tile([128, 2, m], F8, tag=f"x8_{p}", name=f"x8_{p}")
              for p in range(et // 2)]
        w8 = [p0.tile([128, 2, e], F8, tag=f"w8_{p}", name=f"w8_{p}")
              for p in range(et // 2)]

        for rep in range(repeat):
            # per-rep AG buffers: a Shared DRAM buffer may have only one
            # writing instruction, so each rep's AllGather gets its own.
            ag_in = dram.tile([e, m], F8, tag=f"agi{rep}", name=f"agi{rep}")
            ag_out = dram.tile([c, e, m], F8, addr_space="Shared",
                               tag=f"ago{rep}", name=f"ago{rep}")

            # ---------------- Phase 0: projections + row norms -------------
            with (
                tc.tile_pool(name="p0t", bufs=1) as p0t,
                tc.tile_pool(name="psp", bufs=2, space="PSUM") as psp,
                tc.tile_pool(name="pss", bufs=1, space="PSUM") as pss,
            ):
                for s in range(et):
                    nc.sync.dma_start(x8[s // 2][:, s % 2, :],
                                      xt.ap()[s * 128:(s + 1) * 128, :])

                def load_w(wbuf, w_handle):
                    for s in range(et):
                        nc.sync.dma_start(
                            wbuf[s // 2][:, s % 2, :],
                            w_handle.ap()[s * 128:(s + 1) * 128, :],
                        )

                # separate pf/sq sets per projection: lets Q's matmuls emit
                # before K's norm chain without a WAR on K's pf reads. They
                # live in the per-rep pool so etsA can reuse the space later.
                pfs = {nm: [p0t.tile([128, m], F16, tag=f"pf{nm}{t}",
                                     name=f"pf{nm}{t}") for t in range(et)]
                       for nm in "kq"}
                sqs = {nm: [p0t.tile([128, m], F8, tag=f"sq{nm}{t}",
                                     name=f"sq{nm}{t}") for t in range(et)]
                       for nm in "kq"}

                def proj_mm(nm, wbuf, bias_sb, t0=0, t1=None):
                    pf, sq = pfs[nm], sqs[nm]
                    for t in range(t0, et if t1 is None else t1):
                        ps = psp.tile([128, m], F32, tag="pp", name="proj_ps")
                        for sp in range(et // 2):
                            lw = wbuf[sp][:, :, t * 128:(t + 1) * 128]
                            for o, w in mch:
                                nc.tensor.matmul(
                                    ps[:, o:o + w],
                                    lw,
                                    x8[sp][:, :, o:o + w],
                                    start=(sp == 0),
                                    stop=(sp == et // 2 - 1),
                                    perf_mode=PM,
                                )
                        nc.scalar.activation(pf[t], ps, AF.Identity,
                                             bias=bias_sb[:, t:t + 1])
                        nc.vector.tensor_mul(sq[t], pf[t], pf[t])

                def norm_chain(nm, out8, dram_out):
                    # row |.|^2 via ones-matmul; m-chunked so the sqrt ->
                    # recip -> broadcast -> scale chain pipelines per chunk.
                    pf, sq = pfs[nm], sqs[nm]
                    d_ps = pss.tile([1, m], F32, tag="dps", name="d_ps")
                    bc_ps = pss.tile([128, m], F32, tag="bc", name="bc_ps")
                    dsq = p0t.tile([1, m], F32, tag="dsq", name="dsq")
                    rnorm = p0t.tile([1, m], F32, tag="rn", name="rnorm")
                    rn16 = p0t.tile([1, m], F16, tag="rn16", name="rn16")
                    for o, w in mch:
                        for t in range(et):
                            nc.tensor.matmul(
                                d_ps[0:1, o:o + w],
                                ones_k,
                                sq[t][:, o:o + w],
                                start=(t == 0),
                                stop=(t == et - 1),
                            )
                        nc.scalar.activation(dsq[0:1, o:o + w], d_ps[0:1, o:o + w],
                                             AF.Sqrt, bias=eps1[0:1, 0:1])
                        nc.vector.reciprocal(rnorm[0:1, o:o + w], dsq[0:1, o:o + w])
                        nc.vector.tensor_copy(rn16[0:1, o:o + w],
                                              rnorm[0:1, o:o + w])
                        nc.tensor.matmul(
                            bc_ps[:, o:o + w],
                            ones4_m,
                            rn16[0:1, o:o + w],
                        )
                        for t in range(et):
                            nc.vector.tensor_mul(out8(t, o, w), bc_ps[:, o:o + w],
                                                 pf[t][:, o:o + w])
                            if dram_out is not None:
                                nc.sync.dma_start(
                                    dram_out[t * 128:(t + 1) * 128, o:o + w],
                                    out8(t, o, w),
                                )

                kst = [p0t.tile([128, m], F8, tag=f"kst{t % 2}", name=f"kst{t % 2}")
                       for t in range(2)]
                # Interleaved emission: K's norm chain (which feeds the
                # AllGather) is sandwiched between Q matmul blocks so the PE
                # stays busy while the K chain's serial sqrt/recip/broadcast
                # runs, and the AG still launches early.
                load_w(w8, wkt)
                proj_mm("k", w8, bias_k)
                load_w(w8, wqt)
                proj_mm("q", w8, bias_q, 0, 2)
                norm_chain("k", lambda t, o, w: kst[t % 2][:, o:o + w], ag_in)
                proj_mm("q", w8, bias_q, 2)
                norm_chain("q",
                           lambda t, o, w: qn8[t // 2][:, t % 2, o:o + w], None)

            # ---------------- Phase 1: AllGather fp8 Kn^T ------------------
            if not solo:
                nc.gpsimd.collective_compute(
                    "AllGather",
                    mybir.AluOpType.bypass,
                    replica_groups=rg,
                    ins=[ag_in.opt()],
                    outs=[ag_out.opt()],
                )
            if upto < 2:
                continue

            with (
                tc.tile_pool(name="xpp", bufs=4) as xp_pool,
                tc.tile_pool(name="zsb", bufs=2) as z_pool,
                tc.tile_pool(name="psz", bufs=1, space="PSUM") as psz_pool,
                tc.tile_pool(name="kp", bufs=2) as kp_pool,
              ):
               xq_pre = {}

               def load_xq(t, q):
                   xq = xp_pool.tile([128, CH, 128], F8, tag="xq", name="xq")
                   nc.sync.dma_start(xq, xp.ap()[t, :, q * CH:(q + 1) * CH, :])
                   xq_pre[(t, q)] = xq
                   return xq

               def load_kp(cc):
                   kp = [kp_pool.tile([128, 2, m], F8, tag=f"kp{p}", name=f"kp{p}")
                         for p in range(et // 2)]
                   for s in range(et):
                       src_ap = (ag_in[s * 128:(s + 1) * 128, :] if solo
                                 else ag_out[cc, s * 128:(s + 1) * 128, :])
                       nc.sync.dma_start(kp[s // 2][:, s % 2, :], src_ap)
                   return kp

               # prefetch block 0's K^T before the Q-projection's norm-chain
               # work lands on the SP queue: phase 2 can then start the
               # moment qn8 is written.
               kp0 = load_kp(0)

               with (
                tc.tile_pool(name="eA", bufs=1) as eA_pool,
                tc.tile_pool(name="psy", bufs=2, space="PSUM") as psy_pool,
               ):
                etsA = [eA_pool.tile([128, m], F16, tag=f"eA{i}", name=f"eA{i}")
                        for i in range(EB)]

                # ---- Phase 3 helper: AR one chunk of CH n-tiles, then fold
                # v = (exp(y) - cs/8192) * (2^17/cs) into fp8 pair tiles.
                def ar_chunk(k):
                    if upto < 3:
                        return
                    sl = slice(CH * k, CH * (k + 1))
                    ar_in = dram.tile([128, CH], F32, tag=f"ari{rep}_{k}",
                                      name=f"ari{rep}_{k}")
                    ar_out = dram.tile([128, CH], F32, addr_space="Shared",
                                       tag=f"aro{rep}_{k}", name=f"aro{rep}_{k}")
                    nc.sync.dma_start(ar_in, colsum[:, sl])
                    if not solo:
                        nc.gpsimd.collective_compute(
                            "AllReduce",
                            mybir.AluOpType.add,
                            replica_groups=rg,
                            ins=[ar_in.opt()],
                            outs=[ar_out.opt()],
                        )
                    nc.sync.dma_start(cs_full[:, sl], ar_in if solo else ar_out)
                    nc.vector.reciprocal(rec_cs[:, sl], cs_full[:, sl])
                    nc.vector.tensor_scalar_mul(sfac[:, sl], rec_cs[:, sl], S_V)
                    nc.vector.tensor_scalar_mul(tfac[:, sl], cs_full[:, sl],
                                                1.0 / 8192.0)
                    last = (k == nch - 1)
                    for i in range(CH * k, CH * (k + 1)):
                        dst = ets8[i // 2][:, i % 2, :]
                        src_t = etsA[i % EB]
                        # during phase 2 ScalarE is saturated with exp; only
                        # give it affine work on the last chunk. (GPSIMD is
                        # software-emulated on TRN2 -- never put bulk
                        # elementwise work there.)
                        if last and i % 2 == 1:
                            nc.scalar.activation(dst, src_t, AF.Copy,
                                                 scale=sfac[:, i:i + 1], bias=-16.0)
                        else:
                            nc.vector.tensor_scalar(
                                dst, src_t, tfac[:, i:i + 1], sfac[:, i:i + 1],
                                ALU.subtract, ALU.mult)

                # Two psz chains (t=0,1) are interleaved INTO phase 2: the
                # exp stream keeps ScalarE the pacing engine there, so the
                # PE has idle slots for phase-4 work. Emission lags the AR
                # by one chunk so the in-order PE queue never waits on an
                # in-flight AllReduce.
                NI = 2 if upto >= 4 else 0
                zq = nt // CH

                def z_chunk(t, q):
                    xq = xq_pre.pop((t, q), None)
                    if xq is None:
                        xq = load_xq(t, q)
                    psz = psz_live[t]
                    for p2 in range(CH // 2):
                        pair = q * (CH // 2) + p2
                        lw = xq[:, 2 * p2:2 * p2 + 2, :]
                        for o, w in mch:
                            nc.tensor.matmul(
                                psz[:, o:o + w],
                                lw,
                                ets8[pair][:, :, o:o + w],
                                start=(pair == 0),
                                stop=(pair == nt // 2 - 1),
                                perf_mode=PM,
                            )

                psz_live = {}

                # ---------------- Phase 2: Yt via fp8 DoubleRow ------------
                for cc in range(c):
                    kp = kp0 if cc == 0 else load_kp(cc)
                    for j in range(jt):
                        i = cc * jt + j
                        psy = psy_pool.tile([128, m], F32, tag="py", name="psy")
                        for sp in range(et // 2):
                            lw = kp[sp][:, :, j * 128:(j + 1) * 128]
                            for o, w in mch:
                                nc.tensor.matmul(
                                    psy[:, o:o + w],
                                    lw,
                                    qn8[sp][:, :, o:o + w],
                                    start=(sp == 0),
                                    stop=(sp == et // 2 - 1),
                                    perf_mode=PM,
                                )
                        nc.scalar.activation(
                            etsA[i % EB], psy, AF.Exp, scale=1.0 / 16.0,
                            accum_out=colsum[:, i:i + 1],
                        )
                        if i % CH == CH - 1:
                            k = i // CH
                            ar_chunk(k)
                            if NI and k == 0:
                                for t in range(NI):
                                    psz_live[t] = psz_pool.tile(
                                        [128, m], F32, tag=f"pz{t}",
                                        name=f"pz{t}")
                                    load_xq(t, 0)
                            if NI and k >= 1:
                                for t in range(NI):
                                    z_chunk(t, k - 1)

               # ---------------- Phase 4: Zt = X^T @ V (fp8 DoubleRow) -----
               if upto < 4:
                    continue
               for t in range(et):
                    if t < NI:
                        z_chunk(t, zq - 1)  # last chunk of interleaved chain
                    else:
                        psz_live[t] = psz_pool.tile([128, m], F32,
                                                    tag=f"pz{t % 2}",
                                                    name=f"pz{t % 2}")
                        for q in range(zq):
                            z_chunk(t, q)
                    psz = psz_live[t]
                    zsb = z_pool.tile([128, m], F16, tag="zt", name="zsb")
                    nc.scalar.activation(zsb, psz, AF.Identity,
                                         scale=1.0 / S_V, bias=sxb[:, t:t + 1])
                    nc.sync.dma_start(zt.ap()[t * 128:(t + 1) * 128, :], zsb)

    nc.compile()
    return nc


def _prep_inputs(X, Wk, Wq, Wk0, Wq0, n=N, e=E, c=C):
    """Host-side sharding/layout prep. Returns per-core input maps."""
    import concourse.mybir as mybir

    f8 = mybir.dt.np(mybir.dt.float8e4)
    m = n // c
    et = e // 128
    nt = n // 128
    X = np.ascontiguousarray(X, dtype=np.float32)
    wqt = np.ascontiguousarray(np.asarray(Wq, dtype=np.float32).T.astype(f8))
    wkt = np.ascontiguousarray(np.asarray(Wk, dtype=np.float32).T.astype(f8))
    bq = np.ascontiguousarray(Wq0, dtype=np.float32).reshape(et, 128)
    bk = np.ascontiguousarray(Wk0, dtype=np.float32).reshape(et, 128)
    # xp[e_t, p, n_t, cc] = X[n_t*128 + p, e_t*128 + cc], fp8
    xp = np.ascontiguousarray(
        X.astype(f8).reshape(nt, 128, et, 128).transpose(2, 1, 0, 3)
    )
    # exact colsum of X for the centering correction, pre-divided by N
    sx = np.ascontiguousarray(
        (X.astype(np.float64).sum(axis=0) / n).astype(np.float32).reshape(et, 128)
    )
    in_maps = []
    for cc in range(c):
        xt_c = np.ascontiguousarray(X[cc * m:(cc + 1) * m].T.astype(f8))
        in_maps.append(
            {"xt": xt_c, "wqt": wqt, "wkt": wkt, "bq": bq, "bk": bk,
             "xp": xp, "sx": sx}
        )
    return in_maps


def _run(X, Wk, Wq, Wk0, Wq0, trace=False, n=N, e=E, c=C):
    from concourse import bass_utils

    key = (n, e, c)
    if key not in _CACHE:
        _CACHE[key] = _build_program(n, e, c)
    nc = _CACHE[key]
    in_maps = _prep_inputs(X, Wk, Wq, Wk0, Wq0, n, e, c)
    res = bass_utils.run_bass_kernel_spmd(
        nc, in_maps, core_ids=list(range(c)), trace=trace
    )
    m = n // c
    Z = np.empty((n, e), dtype=np.float32)
    for cc in range(c):
        Z[cc * m:(cc + 1) * m, :] = res.results[cc]["zt"].T
    return Z, res


def kernel(X, Wk, Wq, Wk0, Wq0):
    Z, _ = _run(X, Wk, Wq, Wk0, Wq0)
    return Z
